# revision 18
# baseline (speedup 1.0000x reference)
"""Grouped-experts MoE (SwiGLU) Bass kernel for Trainium2, 8 NeuronCores.

Expert-parallel: core c owns experts [8c, 8c+8). Tokens are pre-grouped by
expert in the input, so routing is host-side slicing. All device matmuls run
in transposed-token space so every operand streams in its natural layout:

  gateT[i, t] = sum_k G[k, i] * xT[k, t]      (lhsT = G tile, rhs = xT tile)
  hT = silu(gateT) * upT                       (elementwise, [inter, tok])
  outT[m, t] = sum_ki D[ki, m] * hT[ki, t]     (lhsT = D tile, rhs = hT tile)

v3 scheduling (v1 ~1094us, v2 ~1077us):
  - batched mega-DMAs via 3D access patterns: ONE dma per weight matrix per
    expert and ONE per 512-token x chunk (DMA rings process each dma_start
    serially at ~0.6us issue cost; v2's 16-instruction bursts paced the PE)
  - all weights fully double-buffered (G/U/D tiles never wait on frees ->
    no expert-boundary stalls)
  - per chunk: gate-phase (6 groups) then up-phase (6 groups): U(0) only
    needed ~20us after first matmul -> shorter cold start; silu overlaps
    g-phase, mul overlaps u-phase
  - down-projection groups of chunk t-1 interleaved 2-at-a-time between
    groups of chunk t; outputs copied to bf16 and stored in 4-m-group
    batched DMAs
  - psg has 3 PSUM banks so a slow silu (stuck behind an x DMA issue on the
    scalar queue) can't stall the PE; 3+2+3 = 8 banks used
  - first expert processes its remainder chunk first (smaller cold-start
    footprint); last expert ends with two 128-token chunks (short tail)
Host transposes x in / out once per core (not on the device clock).
Compute in bf16 with fp32 PSUM accumulation; bf16 output.
"""

import numpy as np
import ml_dtypes

NUM_EXPERTS = 64
HID = 2048
INTER = 768
N_CORES = 8
EPC = NUM_EXPERTS // N_CORES  # experts per core
KT = HID // 128    # 16 k-tiles over hidden
IT = INTER // 128  # 6 tiles over intermediate
CHUNK = 512        # moving-operand free dim per matmul (HW max)

BF16_NP = ml_dtypes.bfloat16

_cache = {}


def _chunks_of(p, first_expert, last_expert):
    """Chunk sizes for one expert's p tokens."""
    sizes = []
    full, rem = divmod(p, CHUNK)
    if first_expert:
        if rem:
            sizes.append(rem)
        sizes += [CHUNK] * full
    else:
        sizes += [CHUNK] * full
        if rem:
            sizes.append(rem)
    out = []
    n0 = 0
    for s in sizes:
        out.append((n0, s))
        n0 += s
    return out


def _chunk_list(padded):
    """[(slot, col0, n, first_of_expert)] over all experts of this core."""
    ch = []
    off = 0
    ne = len(padded)
    for j, p in enumerate(padded):
        for idx, (n0, n) in enumerate(
                _chunks_of(p, j == 0, j == ne - 1)):
            ch.append((j, off + n0, n, idx == 0))
        off += p
    return ch


def _build(padded):
    import concourse.bacc as bacc
    import concourse.mybir as mybir
    import concourse.tile as tile

    BF16 = mybir.dt.bfloat16
    F32 = mybir.dt.float32
    SILU = mybir.ActivationFunctionType.Silu

    ptot = int(sum(padded))
    CH = _chunk_list(padded)
    NCH = len(CH)

    nc = bacc.Bacc("TRN2", target_bir_lowering=False, debug=False,
                   num_devices=N_CORES)

    xt = nc.dram_tensor("xt", [HID, ptot], BF16, kind="ExternalInput")
    gw = nc.dram_tensor("gw", [EPC, HID, INTER], BF16, kind="ExternalInput")
    uw = nc.dram_tensor("uw", [EPC, HID, INTER], BF16, kind="ExternalInput")
    dw = nc.dram_tensor("dw", [EPC, INTER, HID], BF16, kind="ExternalInput")
    yt = nc.dram_tensor("yt", [HID, ptot], BF16, kind="ExternalOutput")

    with tile.TileContext(nc) as tc:
        with (
            tc.tile_pool(name="xp", bufs=2) as xp,     # 2 x 16K/part
            tc.tile_pool(name="gp", bufs=2) as gp,     # 2 x 24K
            tc.tile_pool(name="upl", bufs=2) as upl,   # 2 x 24K
            tc.tile_pool(name="dp", bufs=2) as dp,     # 2 x 24K
            tc.tile_pool(name="hp", bufs=12) as hp,    # 12K
            tc.tile_pool(name="sp", bufs=12) as sp,    # 12K (bf16)
            tc.tile_pool(name="op", bufs=3) as op,     # 3 x 2K
            tc.tile_pool(name="psg", bufs=3, space="PSUM") as psg,
            tc.tile_pool(name="psu", bufs=2, space="PSUM") as psu,
            tc.tile_pool(name="psd", bufs=3, space="PSUM") as psd,
        ):
            gt = {}   # expert -> [128, KT, INTER] tile
            ut = {}
            dk = {}   # expert -> [128, IT, HID] tile
            xtl = {}  # chunk idx -> [128, KT, n] tile

            def gsrc(e):
                return gw[e].rearrange("(k p) i -> p k i", p=128)

            def usrc(e):
                return uw[e].rearrange("(k p) i -> p k i", p=128)

            def dsrc(e):
                return dw[e].rearrange("(k p) m -> p k m", p=128)

            def load_weights(e, queue):
                g = gp.tile([128, KT, INTER], BF16, tag="g", name=f"g{e}")
                queue.dma_start(g[:], gsrc(e))
                gt[e] = g
                u = upl.tile([128, KT, INTER], BF16, tag="u", name=f"u{e}")
                queue.dma_start(u[:], usrc(e))
                ut[e] = u
                d = dp.tile([128, IT, HID], BF16, tag="d", name=f"d{e}")
                queue.dma_start(d[:], dsrc(e))
                dk[e] = d

            def load_x(t, queue):
                slot, col0, n, _ = CH[t]
                x = xp.tile([128, KT, n], BF16, tag="x", name=f"x{t}",
                            padded_shape=[128, KT, CHUNK])
                queue.dma_start(
                    x[:], xt[:, col0:col0 + n].rearrange(
                        "(k p) n -> p k n", p=128))
                xtl[t] = x

            # ---- cold-start prologue ----
            # The big batched DMAs (one per matrix) are ring-serial and too
            # slow for the critical first expert. Use per-k-tile DMAs spread
            # round-robin over all three trigger queues so many rings run in
            # parallel, ordered by need: G0+x0 first, then U0, x1, D0.
            QS = [nc.sync, nc.scalar, nc.gpsimd]
            q = 0

            def spread(dst, src, ranges):
                # ~1MB pieces round-robined over the queues: big enough to
                # amortize the ~0.6us per-dma ring issue cost, small enough
                # that no single ring serializes a critical tensor.
                nonlocal q
                for a, b in ranges:
                    QS[q % 3].dma_start(dst[:, a:b, :], src[:, a:b, :])
                    q += 1

            R6 = [(0, 3), (3, 6), (6, 9), (9, 12), (12, 14), (14, 16)]
            R3 = [(0, 6), (6, 11), (11, 16)]

            # Strict need-order: g-phase(c0) needs G0+x0 (~13us), g-phase(c1)
            # needs x1 (~21us), u-phase(c0) needs U0 (~34us), downs(c0) need
            # D0 (~45us).
            g0 = gp.tile([128, KT, INTER], BF16, tag="g", name="g0")
            slot0, col0_0, n_0, _ = CH[0]
            x0 = xp.tile([128, KT, n_0], BF16, tag="x", name="x0",
                         padded_shape=[128, KT, CHUNK])
            spread(g0, gsrc(0), R6)
            spread(x0, xt[:, col0_0:col0_0 + n_0].rearrange(
                "(k p) n -> p k n", p=128), R3)
            gt[0] = g0
            xtl[0] = x0
            slot1, col0_1, n_1, _ = CH[1]
            x1 = xp.tile([128, KT, n_1], BF16, tag="x", name="x1",
                         padded_shape=[128, KT, CHUNK])
            spread(x1, xt[:, col0_1:col0_1 + n_1].rearrange(
                "(k p) n -> p k n", p=128), R3)
            xtl[1] = x1
            u0 = upl.tile([128, KT, INTER], BF16, tag="u", name="u0")
            spread(u0, usrc(0), R6)
            ut[0] = u0
            d0 = dp.tile([128, IT, HID], BF16, tag="d", name="d0")
            spread(d0, dsrc(0), [(0, 2), (2, 4), (4, 6)])
            dk[0] = d0

            h = {}          # (chunk, i) -> h tile
            pend = None     # chunk whose down-groups still need emitting
            emitted = 0
            out_tiles = {}  # (chunk, mblk) -> batched output tile

            def down_group(t, m):
                slot, col0, n, _ = CH[t]
                e = slot
                pd = psd.tile([128, n], F32, tag="pd",
                              padded_shape=[128, CHUNK])
                for ki in range(IT):
                    nc.tensor.matmul(pd[:],
                                     dk[e][:, ki, m * 128:(m + 1) * 128],
                                     h[(t, ki)][:],
                                     start=(ki == 0), stop=(ki == IT - 1))
                blk = m // 2
                if m % 2 == 0:
                    out_tiles[(t, blk)] = op.tile(
                        [128, 2, n], BF16, tag="o", name=f"o{t}_{blk}",
                        padded_shape=[128, 2, CHUNK])
                ot = out_tiles[(t, blk)]
                nc.vector.tensor_copy(ot[:, m % 2, :], pd[:])
                if m % 2 == 1:
                    nc.gpsimd.dma_start(
                        yt[blk * 256:(blk + 1) * 256,
                           col0:col0 + n].rearrange(
                               "(g p) n -> p g n", p=128),
                        ot[:])
                    del out_tiles[(t, blk)]

            def emit_downs(upto):
                nonlocal emitted
                if pend is None:
                    return
                while emitted < upto:
                    down_group(pend, emitted)
                    emitted += 1

            # down-groups of chunk t-1 emitted after each group of chunk t:
            # 2 after each gate group g(1)..g(5), 2 after each up group
            # u(0)..u(2)  -> 16 total
            G_SCHED = [0, 2, 4, 6, 8, 10]
            U_SCHED = [12, 14, 16, 16, 16, 16]

            sts = {}  # (chunk, i) -> silu tile

            def g_phase(t, sched=None):
                slot, col0, n, _ = CH[t]
                e = slot
                for i in range(IT):
                    pg = psg.tile([128, n], F32, tag="pg",
                                  padded_shape=[128, CHUNK])
                    for k in range(KT):
                        nc.tensor.matmul(pg[:],
                                         gt[e][:, k, i * 128:(i + 1) * 128],
                                         xtl[t][:, k, :],
                                         start=(k == 0), stop=(k == KT - 1))
                    st = sp.tile([128, n], BF16, tag="s", name=f"s{t}_{i}",
                                 padded_shape=[128, CHUNK])
                    nc.scalar.activation(st[:], pg[:], SILU)
                    sts[(t, i)] = st
                    if sched is not None:
                        emit_downs(sched[i])

            def u_phase(t, sched=None):
                slot, col0, n, _ = CH[t]
                e = slot
                for i in range(IT):
                    pu = psu.tile([128, n], F32, tag="pu",
                                  padded_shape=[128, CHUNK])
                    for k in range(KT):
                        nc.tensor.matmul(pu[:],
                                         ut[e][:, k, i * 128:(i + 1) * 128],
                                         xtl[t][:, k, :],
                                         start=(k == 0), stop=(k == KT - 1))
                    ht = hp.tile([128, n], BF16, tag="h",
                                 padded_shape=[128, CHUNK])
                    nc.vector.tensor_mul(ht[:], sts.pop((t, i))[:], pu[:])
                    h[(t, i)] = ht
                    if sched is not None:
                        emit_downs(sched[i])

            def finish_chunk(t):
                # x prefetch AFTER this chunk's scalar-queue ops: the DMA may
                # wait on the x(t) buffer free (u-group(t,5)); emitting it
                # earlier would head-of-line block silus -> deadlock against
                # the PSUM-bank WAR dependency.
                nonlocal pend, emitted
                if t + 2 < NCH:
                    load_x(t + 2, nc.scalar)
                emit_downs(KT)
                if pend is not None:
                    for ki in range(IT):
                        del h[(pend, ki)]
                pend = t
                emitted = 0

            t0 = 0
            defer_w1 = False
            if NCH >= 2 and CH[1][0] == CH[0][0]:
                # Cold-start special case: interleave the first two chunks'
                # phases so the PE has gate work (needs only G0+x) while U0
                # is still streaming in.
                t0 = 2
                # Defer expert-1's weight prefetch past the cold window when
                # possible (it would steal HBM bandwidth from the critical
                # U0/x1/D0 arrivals): ride the scalar queue behind the next
                # chunk's silus. Only safe if chunk t0 still belongs to
                # expert 0.
                defer_w1 = EPC > 1 and t0 < NCH and CH[t0][0] == 0
                if EPC > 1 and not defer_w1:
                    load_weights(1, nc.sync)
                g_phase(0)
                g_phase(1)
                u_phase(0)
                if NCH > 2:
                    load_x(2, nc.scalar)
                pend, emitted = 0, 0
                u_phase(1, sched=[4, 8, 12, 16, 16, 16])  # downs of chunk 0
                finish_chunk(1)  # x(3) prefetch; drains pend 0; pend=1

            for t in range(t0, NCH):
                slot, col0, n, first = CH[t]
                if first and slot + 1 < EPC:
                    load_weights(slot + 1, nc.sync)
                g_phase(t, sched=G_SCHED)
                if defer_w1 and t == t0:
                    load_weights(1, nc.scalar)
                    defer_w1 = False
                u_phase(t, sched=U_SCHED)
                finish_chunk(t)

            # tail: down-groups of the final chunk
            for m in range(KT):
                down_group(pend, m)

    nc.compile()
    return nc, ptot


def _get_program(padded):
    key = tuple(padded)
    if key not in _cache:
        _cache[key] = _build(padded)
    return _cache[key]


def _invoke(x, gate_proj, up_proj, down_proj, num_tokens_per_expert,
            trace=False, trace_kwargs=None):
    from concourse.bass_utils import run_bass_kernel_spmd

    x = np.asarray(x)
    counts = np.asarray(num_tokens_per_expert).astype(np.int64)
    assert counts.shape == (NUM_EXPERTS,)
    starts = np.zeros(NUM_EXPERTS + 1, dtype=np.int64)
    np.cumsum(counts, out=starts[1:])

    # per-slot padded counts (max over cores) -> one SPMD program
    cmat = counts.reshape(N_CORES, EPC)
    padded = [int(cmat[:, j].max()) for j in range(EPC)]
    offs = np.zeros(EPC + 1, dtype=np.int64)
    np.cumsum(np.asarray(padded), out=offs[1:])
    ptot_expected = int(offs[-1])

    nc, ptot = _get_program(padded)
    assert ptot == ptot_expected

    gb = np.asarray(gate_proj).astype(BF16_NP)
    ub = np.asarray(up_proj).astype(BF16_NP)
    db = np.asarray(down_proj).astype(BF16_NP)

    in_maps = []
    for c in range(N_CORES):
        xtc = np.zeros((HID, ptot), dtype=BF16_NP)
        for j in range(EPC):
            e = c * EPC + j
            cnt = int(counts[e])
            if cnt:
                xtc[:, int(offs[j]):int(offs[j]) + cnt] = \
                    x[int(starts[e]):int(starts[e]) + cnt].astype(BF16_NP).T
        in_maps.append({
            "xt": xtc,
            "gw": gb[c * EPC:(c + 1) * EPC],
            "uw": ub[c * EPC:(c + 1) * EPC],
            "dw": db[c * EPC:(c + 1) * EPC],
        })

    res = run_bass_kernel_spmd(nc, in_maps, list(range(N_CORES)),
                               trace=trace, **(trace_kwargs or {}))

    out = np.empty((int(starts[-1]), HID), dtype=np.float32)
    for c in range(N_CORES):
        ytc = res.results[c]["yt"]
        for j in range(EPC):
            e = c * EPC + j
            cnt = int(counts[e])
            if cnt:
                out[int(starts[e]):int(starts[e]) + cnt] = \
                    ytc[:, int(offs[j]):int(offs[j]) + cnt].T \
                    .astype(np.float32)
    return out, res


def kernel(x, gate_proj, up_proj, down_proj, num_tokens_per_expert):
    out, _ = _invoke(x, gate_proj, up_proj, down_proj, num_tokens_per_expert)
    return out
